# revision 32
# baseline (speedup 1.0000x reference)
"""Trainium2 Bass kernel for the delta-rule memory recurrence (DeltaNet-style).

Full-input contract: kernel(memory, key, value) -> final memory, all np.ndarray,
shapes (16,256,256), (16,4096,256), (16,4096,256) -> (16,256,256) float32.

Strategy: pure data-parallel over batch (2 batches per NeuronCore x 8 cores).
Per batch the sequential recurrence

    kn   = k_t / ||k_t||
    M   <- M - (1.1 * M kn - 0.1 * v_t) kn^T

is reformulated chunkwise (C=128 steps per chunk) via the WY / UT transform:

    A  = Kn Kn^T                      (C x C Gram of normalized keys)
    L  = 1.1 * strict_lower(A)
    Tinv = (I + L)^{-1}               (unit lower triangular inverse)
    H  = Tinv @ (-1.1 * Kn Mt + 0.1 * V)
    Mt <- Mt + Kn^T H                 (Mt = M^T state, (DK, DV))

(I+L)^{-1} is computed exactly with the nilpotent factorization
(I-L)(I+L^2)(I+L^4)(I+L^8)  [L^16 and beyond are numerically zero here].
Inversion machinery runs in fp16 matmuls (full PE rate, 10-bit mantissa),
state-path matmuls run as float32r (full rate at N>=256).
"""

import numpy as np

import concourse.bass as bass
import concourse.mybir as mybir
import concourse.tile as tile
from concourse.bass import ts
from concourse.bass_utils import run_bass_kernel_spmd
from concourse.masks import make_identity

F32 = mybir.dt.float32
F32R = mybir.dt.float32r
F16 = mybir.dt.float16
AOP = mybir.AluOpType
AFT = mybir.ActivationFunctionType

B, S, DK, DV = 16, 4096, 256, 256
NCORES = 8
BLOC = B // NCORES          # batches per core
C = 128                     # chunk length
LR = 0.1
AC = 1.0 + LR               # 1.1
NLEV = 3                    # squaring levels: (I-L)(I+L^2)(I+L^4)(I+L^8)


def _split_waits(nc, max_waits=1):
    """walrus codegen on this toolchain encodes at most one semaphore wait per
    instruction; hoist excess waits onto same-engine NoOps placed just before."""
    n_split = 0
    for f in nc.m.functions:
        for bb in f.blocks:
            insts = bb.instructions
            out = []
            for inst in insts:
                si = getattr(inst, "sync_info", None)
                w = list(si.on_wait) if (si and si.on_wait) else []
                k = 0
                while len(w) > max_waits:
                    head, w = w[:max_waits], w[max_waits:]
                    out.append(mybir.InstNoOp(
                        name=f"{inst.name}-wsplit{k}",
                        engine=inst.engine,
                        sync_info=mybir.SyncInfo(on_wait=head, on_update=[]),
                    ))
                    n_split += 1
                    k += 1
                if k:
                    inst.sync_info = mybir.SyncInfo(
                        on_wait=w, on_update=list(si.on_update or [])
                    )
                out.append(inst)
            bb.instructions = out
    return n_split


def build_nc(s_loc=S, state_mm_dtype=F32R, split=True):
    nch = s_loc // C
    nc = bass.Bass()
    memT = nc.declare_dram_parameter("memT", [BLOC, DK, DV], F32, isOutput=False)
    key_d = nc.declare_dram_parameter("key", [BLOC, s_loc, DK], F32, isOutput=False)
    val_d = nc.declare_dram_parameter("value", [BLOC, s_loc, DV], F32, isOutput=False)
    outT = nc.declare_dram_parameter("outT", [BLOC, DK, DV], F32, isOutput=True)

    SMM = state_mm_dtype  # state-path matmul tiles (float32r: full-rate fp32-ish mm)

    with tile.TileContext(nc) as tc:
        with (
            tc.tile_pool(name="consts", bufs=1) as consts,
            tc.tile_pool(name="scr", bufs=4) as scrp,
            tc.tile_pool(name="kv", bufs=10) as kv,
            tc.tile_pool(name="vv", bufs=16) as vv,
            tc.tile_pool(name="norm", bufs=12) as normp,
            tc.tile_pool(name="kt", bufs=11) as ktp,
            tc.tile_pool(name="inv", bufs=8) as invp,
            tc.tile_pool(name="state", bufs=4) as statep,
            tc.tile_pool(name="mt", bufs=3) as mtp,
            tc.tile_pool(name="mtinit", bufs=1) as mtinitp,
            tc.tile_pool(name="ps_inv", bufs=4, space="PSUM") as ps_inv,
            tc.tile_pool(name="ps_state", bufs=2, space="PSUM") as ps_state,
            tc.tile_pool(name="ps_mt0", bufs=1, space="PSUM") as ps_mt0,
            tc.tile_pool(name="ps_mt1", bufs=1, space="PSUM") as ps_mt1,
        ):
            one_reg = nc.gpsimd.to_reg(1.0)
            ident32 = consts.tile([128, 128], F32, tag="ident32")
            make_identity(nc, ident32)
            ident16 = consts.tile([128, 128], F16, tag="ident16")
            make_identity(nc, ident16)
            # paired identity (both halves) for G0 = I + LTn
            i2_16 = consts.tile([128, 2, 128], F16, tag="i2_16")
            nc.gpsimd.memset(i2_16, 0.0)
            nc.gpsimd.affine_select(
                out=i2_16, in_=i2_16, compare_op=AOP.not_equal, fill=1.0,
                base=0, pattern=[[0, 2], [-1, 128]], channel_multiplier=1,
            )

            # state Mt (= M^T) per batch lives in PSUM and accumulates the
            # per-chunk updates; an SBUF f32r copy is refreshed each chunk.
            # Initial value injected via exact fp32 identity-matmul.
            mt = []
            mt_ps = []
            for b, pool in ((0, ps_mt0), (1, ps_mt1)):
                t0 = mtinitp.tile([128, 2, DV], F32, tag=f"mt0f{b}")
                nc.sync.dma_start(
                    out=t0, in_=memT[b].rearrange("(j p) v -> p j v", p=128)
                )
                ps = pool.tile([128, 2, DV], F32, tag=f"mtps{b}")
                # one matmul over the whole [128, 512] bank: a second
                # start=True would clear the first slice's has_written bits
                nc.tensor.matmul(ps.rearrange("p j v -> p (j v)"), ident32,
                                 t0.rearrange("p j v -> p (j v)"),
                                 start=True, stop=False,
                                 skip_group_check=True)
                t = mtp.tile([128, 2, DV], SMM, tag=f"mt{b}")
                nc.vector.tensor_copy(t, ps)
                mt.append(t)
                mt_ps.append(ps)

            def cp(dst, src_ap, b, scale=None):
                """psum->sbuf copy of one batch slice; b0 -> DVE, b1 -> ACT."""
                if b == 0:
                    if scale is None:
                        nc.vector.tensor_copy(dst, src_ap)
                    else:
                        nc.vector.tensor_scalar_mul(dst, src_ap, scale)
                else:
                    if scale is None:
                        nc.scalar.copy(dst, src_ap)
                    else:
                        nc.scalar.mul(dst, src_ap, scale)

            def emit_precomp_batch(cs, A=None, phase=0):
                """Stage-major precompute for several chunks: each stage runs
                across all chunks back-to-back so the PE stream stays dense.
                phase 1 = loads..masks, phase 2 = power/G chains, 0 = both."""
                if A is None:
                    A = [dict(c=c) for c in cs]
                if phase == 2:
                    return emit_precomp_phase2(A)
                for a in A:                       # loads
                    c = a["c"]
                    a["Kt"], a["Vt"] = [], []
                    for b in range(BLOC):
                        k = kv.tile([128, DK], F32, tag=f"k{b}")
                        nc.sync.dma_start(out=k,
                                          in_=key_d[b, c * C:(c + 1) * C, :])
                        v = vv.tile([128, DV], F32, tag=f"v{b}")
                        nc.sync.dma_start(out=v,
                                          in_=val_d[b, c * C:(c + 1) * C, :])
                        a["Kt"].append(k)
                        a["Vt"].append(v)
                for a in A:                       # normalization
                    a["Kn"] = []
                    ssq = normp.tile([128, 2], F32, tag="ssq")
                    for b in range(BLOC):
                        scr = scrp.tile([128, DK], F16, tag="scr")
                        if b == 0:
                            nc.vector.scalar_tensor_tensor(
                                out=scr, in0=a["Kt"][b], scalar=1.0,
                                in1=a["Kt"][b], op0=AOP.mult, op1=AOP.mult,
                                accum_out=ssq[:, b:b + 1])
                        else:
                            nc.scalar.activation(out=scr, in_=a["Kt"][b],
                                                 func=AFT.Square,
                                                 accum_out=ssq[:, b:b + 1])
                    nrm = normp.tile([128, 2], F32, tag="nrm")
                    nc.scalar.activation(nrm, ssq, AFT.Sqrt)
                    rn = normp.tile([128, 2], F32, tag="rn")
                    nc.vector.reciprocal(rn, nrm)
                    for b in range(BLOC):
                        kn = normp.tile([128, DK], SMM, tag=f"kn{b}")
                        if b == 0:
                            nc.vector.tensor_scalar_mul(kn, a["Kt"][b],
                                                        rn[:, 0:1])
                        else:
                            nc.scalar.activation(kn, a["Kt"][b], AFT.Copy,
                                                 scale=rn[:, 1:2])
                        a["Kn"].append(kn)
                for a in A:                       # transposes
                    a["KnTs"] = []
                    for j in range(2):
                        tp = ps_inv.tile([128, 2, 128],
                                         F16 if SMM == F16 else F32, tag="inv")
                        for b in range(BLOC):
                            src_ap = a["Kn"][b][:, ts(j, 128)]
                            if SMM != F16:
                                src_ap = src_ap.bitcast(F32)
                            nc.tensor.transpose(
                                tp[:, b, :], src_ap,
                                ident16 if SMM == F16 else ident32)
                        s32 = ktp.tile([128, 2, 128], SMM, tag=f"knts{j}")
                        cp(s32, tp, j)      # j=0 -> DVE, j=1 -> ACT
                        a["KnTs"].append(s32)
                for a in A:                       # Gram matrix + masks
                    a_ps = ps_inv.tile([128, 2, 128], F32, tag="inv")
                    for b in range(BLOC):
                        for j in range(2):
                            nc.tensor.matmul(
                                a_ps[:, b, :], a["KnTs"][j][:, b, :],
                                a["KnTs"][j][:, b, :],
                                start=(j == 0), stop=(j == 1),
                            )
                    a_neg = invp.tile([128, 2, 128], F16, tag="a_neg")
                    cp(a_neg, a_ps, 1, scale=-AC)      # ACT
                    a["a_neg"] = a_neg
                for a in A:                       # triangular masks (gpsimd)
                    ln = invp.tile([128, 2, 128], F16, tag="ln")
                    ltn = invp.tile([128, 2, 128], F16, tag="ltn")
                    g0 = invp.tile([128, 2, 128], F16, tag="g0")
                    for b in range(BLOC):
                        nc.gpsimd.affine_select(
                            out=ln[:, b, :], in_=a["a_neg"][:, b, :],
                            compare_op=AOP.is_gt, fill=0.0,
                            base=0, pattern=[[-1, 128]], channel_multiplier=1,
                        )
                        nc.gpsimd.affine_select(
                            out=ltn[:, b, :], in_=a["a_neg"][:, b, :],
                            compare_op=AOP.is_gt, fill=0.0,
                            base=0, pattern=[[1, 128]], channel_multiplier=-1,
                        )
                        # G0 = I - LT: strict upper of a_neg, diagonal = 1
                        nc.gpsimd.affine_select(
                            out=g0[:, b, :], in_=a["a_neg"][:, b, :],
                            compare_op=AOP.is_gt, fill=0.0,
                            base=0, pattern=[[1, 128]], channel_multiplier=-1,
                        )
                        nc.gpsimd.affine_select(
                            out=g0[:, b, :], in_=g0[:, b, :],
                            compare_op=AOP.not_equal, fill=one_reg,
                            base=0, pattern=[[-1, 128]], channel_multiplier=1,
                        )
                    a["ln"], a["ltn"] = ln, ltn
                    a["g"] = g0
                if phase == 1:
                    return A
                return emit_precomp_phase2(A)

            def emit_precomp_phase2(A):
                def pow_pair(a, lhsT_l, rhs_l, lhsT_lt, rhs_lt, tag):
                    ps = ps_inv.tile([128, 2, 256], F32, tag="inv")
                    for b in range(BLOC):
                        nc.tensor.matmul(ps[:, b, 0:128],
                                         lhsT_l[:, b, :], rhs_l[:, b, :])
                        nc.tensor.matmul(ps[:, b, 128:256],
                                         lhsT_lt[:, b, :], rhs_lt[:, b, :])
                    sb = invp.tile([128, 2, 256], F16, tag=tag)
                    cp(sb, ps, 0 if tag == "p2" else 1)
                    return sb[:, :, 0:128], sb[:, :, 128:256]

                for a in A:                       # L^2 / L^2T
                    a["l2"], a["lt2"] = pow_pair(a, a["ltn"], a["ln"],
                                                 a["ln"], a["ltn"], "p2")
                for a in A:                       # L^4 / L^4T
                    a["l4"], a["lt4"] = pow_pair(a, a["lt2"], a["l2"],
                                                 a["l2"], a["lt2"], "p4")
                for a in A:                       # G1 = (I + LT2) G0
                    gp = ps_inv.tile([128, 2, 128], F32, tag="inv")
                    gn = invp.tile([128, 2, 128], F16, tag="g1")
                    for b in range(BLOC):
                        nc.tensor.matmul(gp[:, b, :], a["l2"][:, b, :],
                                         a["g"][:, b, :],
                                         start=True, stop=False)
                        nc.tensor.matmul(gp[:, b, :], ident16, a["g"][:, b, :],
                                         start=False, stop=True)
                    cp(gn, gp, 1)                 # ACT
                    a["g"] = gn
                for a in A:                       # L^8
                    p8 = ps_inv.tile([128, 2, 128], F32, tag="inv")
                    for b in range(BLOC):
                        nc.tensor.matmul(p8[:, b, :], a["lt4"][:, b, :],
                                         a["l4"][:, b, :])
                    l8 = invp.tile([128, 2, 128], F16, tag="p8")
                    cp(l8, p8, 1)                 # ACT
                    a["l8"] = l8
                for a in A:                       # G2 = (I + LT4) G1 (DVE add)
                    gp = ps_inv.tile([128, 2, 128], F32, tag="inv")
                    gn = invp.tile([128, 2, 128], F16, tag="g2")
                    for b in range(BLOC):
                        nc.tensor.matmul(gp[:, b, :], a["l4"][:, b, :],
                                         a["g"][:, b, :])
                    for b in range(BLOC):
                        nc.vector.tensor_add(gn[:, b, :], a["g"][:, b, :],
                                             gp[:, b, :])
                    a["g"] = gn
                for a in A:                       # G3 = (I + LT8) G2
                    gp = ps_inv.tile([128, 2, 128], F32, tag="inv")
                    gn = invp.tile([128, 2, 128], F16, tag="g3")
                    for b in range(BLOC):
                        nc.tensor.matmul(gp[:, b, :], a["l8"][:, b, :],
                                         a["g"][:, b, :],
                                         start=True, stop=False)
                        nc.tensor.matmul(gp[:, b, :], ident16, a["g"][:, b, :],
                                         start=False, stop=True)
                    cp(gn, gp, 0)                 # DVE
                    a["g"] = gn
                return A

            def emit_state(art):
                Kn, Vt, KnTs, g = art["Kn"], art["Vt"], art["KnTs"], art["g"]
                last = art["c"] == nch - 1
                y_ps, rh, h_ps, h_sb = [], [], [], []
                for b in range(BLOC):
                    y = ps_state.tile([128, DV], F32, tag="st")
                    for j in range(2):
                        nc.tensor.matmul(
                            y, KnTs[j][:, b, :], mt[b][:, j, :],
                            start=(j == 0), stop=(j == 1),
                        )
                    y_ps.append(y)
                for b in range(BLOC):
                    # R' = 10*R = -11 Kn Mt + V  (fp16); the 0.1 folds into H
                    r = statep.tile([128, DV], F16, tag=f"rh{b}")
                    nc.vector.scalar_tensor_tensor(
                        out=r, in0=y_ps[b], scalar=-10.0 * AC, in1=Vt[b],
                        op0=AOP.mult, op1=AOP.add,
                    )
                    rh.append(r)
                for b in range(BLOC):
                    h = ps_state.tile([128, DV], F32, tag="st")
                    nc.tensor.matmul(h, g[:, b, :], rh[b])
                    h_ps.append(h)
                for b in range(BLOC):
                    h = statep.tile([128, DV], SMM, tag=f"hs{b}")
                    cp(h, h_ps[b], b, scale=LR)        # H = 0.1 * Tinv R'
                    h_sb.append(h)
                for b in range(BLOC):
                    for j in range(2):
                        nc.tensor.matmul(
                            mt_ps[b][:, j, :], Kn[b][:, ts(j, 128)], h_sb[b],
                            start=False, stop=last, skip_group_check=True,
                        )
                for b in range(BLOC):
                    mt_new = mtp.tile([128, 2, DV], SMM, tag=f"mt{b}")
                    cp(mt_new, mt_ps[b], b)
                    mt[b] = mt_new

            # software pipeline, super-batched: the state-independent
            # precompute for the NEXT group of chunks is emitted stage-major
            # (dense independent matmul streams) before this group's
            # sequential state chain.
            SB = 4
            groups = [list(range(i, min(i + SB, nch))) for i in range(0, nch, SB)]
            arts = emit_precomp_batch(groups[0])
            ph1next = (emit_precomp_batch(groups[1], phase=1)
                       if len(groups) > 1 else None)
            for gi, grp in enumerate(groups):
                nxt = (emit_precomp_batch(groups[gi + 1], A=ph1next, phase=2)
                       if ph1next is not None else None)
                half = (len(arts) + 1) // 2
                for art in arts[:half]:
                    emit_state(art)
                ph1next = (emit_precomp_batch(groups[gi + 2], phase=1)
                           if gi + 2 < len(groups) else None)
                for art in arts[half:]:
                    emit_state(art)
                arts = nxt

            for b in range(BLOC):
                fin = mtinitp.tile([128, 2, DV], F32, tag=f"fin{b}")
                nc.vector.tensor_copy(fin, mt_ps[b])
                nc.sync.dma_start(
                    out=outT[b].rearrange("(j p) v -> p j v", p=128),
                    in_=fin,
                )
    if split:
        _split_waits(nc)
    return nc


_NC_CACHE = {}

# test-harness hooks (the grading harness just calls kernel())
TRACE = False
LAST_RESULT = None
STATE_DT = F16


def _get_nc(s_loc=S):
    key = (s_loc, STATE_DT)
    if key not in _NC_CACHE:
        _NC_CACHE[key] = build_nc(s_loc, state_mm_dtype=STATE_DT)
    return _NC_CACHE[key]


def kernel(memory, key, value):
    global LAST_RESULT
    memory = np.ascontiguousarray(np.asarray(memory), dtype=np.float32)
    key = np.ascontiguousarray(np.asarray(key), dtype=np.float32)
    value = np.ascontiguousarray(np.asarray(value), dtype=np.float32)
    s_loc = key.shape[1]
    nc = _get_nc(s_loc)
    memT = np.ascontiguousarray(memory.transpose(0, 2, 1))
    in_maps = []
    for i in range(NCORES):
        sl = slice(i * BLOC, (i + 1) * BLOC)
        in_maps.append({
            "memT": memT[sl],
            "key": np.ascontiguousarray(key[sl]),
            "value": np.ascontiguousarray(value[sl]),
        })
    res = run_bass_kernel_spmd(nc, in_maps, list(range(NCORES)), trace=TRACE)
    LAST_RESULT = res
    outs = [res.results[i]["outT"] for i in range(NCORES)]
    out = np.concatenate(outs, axis=0)          # (16, DK, DV) = M^T
    return np.ascontiguousarray(out.transpose(0, 2, 1))
